# revision 9
# baseline (speedup 1.0000x reference)
"""Trainium2 Bass kernel for the note/wiki 3-way contraction + gate MLP.

Math (per note n):
    e[n]    = (wikivec * notevec[n]) @ W_emb.T + b_emb          # (C, K)
    attn[n] = sigmoid(e[n] @ W_att.T + b_att)                   # (C, K)
    s[n]    = sum_k attn[n]*e[n]*W_out[0,k] + b_out             # (C,)

Sharding: data-parallel over the 16 notes -> 2 notes per core on 8 cores.
wikivec / W_emb are replicated (pre-transposed, zero-padded to 10112 = 79*128
along the contraction axis, cast to bf16 on the host so the per-core HBM->SBUF
stream is ~10 MB and hides under the PE work).

Device layout (all v-major so the contraction dim sits on partitions):
  phase 1: for each of 79 v-tiles, scale wikivec^T[v,:] by notevec[n,v]
           (per-partition scalar; note0 on DVE, note1 on ACT) into one
           [128, 512] bf16 moving tile, then 2 matmuls (k-halves) accumulate
           e^T[k, (note,c)] into two PSUM banks over all 79 v-tiles.
  phase 2: bias via ACT Identity, bf16 copy, 4 matmuls for attn logits,
           sigmoid, gate, W_out contraction, + b_out, DMA out s [1, 512].
"""

import sys

if "/opt/trn_rl_repo" not in sys.path:
    sys.path.insert(0, "/opt/trn_rl_repo")

import numpy as np
import ml_dtypes

import concourse.bass as bass
import concourse.mybir as mybir
import concourse.tile as tile
from concourse import bacc
from concourse.bass_utils import run_bass_kernel_spmd

N_CORES = 8
N, C, V, K = 16, 256, 10000, 256
J = 79  # number of 128-row v-tiles (V padded to 10112)
VP = J * 128
NLOC = N // N_CORES  # notes per core

F32 = mybir.dt.float32
BF16 = mybir.dt.bfloat16
BF16_NP = ml_dtypes.bfloat16

_NC_CACHE = {}


def _build_nc():
    nc = bacc.Bacc(None, target_bir_lowering=False)

    wikiT = nc.declare_dram_parameter("wikiT", [J, 128, C], BF16, isOutput=False)
    wembT = nc.declare_dram_parameter("wembT", [J, 128, K], BF16, isOutput=False)
    scales = nc.declare_dram_parameter("scales", [128, NLOC * J], F32, isOutput=False)
    watT = nc.declare_dram_parameter("watT", [2, 128, K], BF16, isOutput=False)
    woutT = nc.declare_dram_parameter("woutT", [128, 2], F32, isOutput=False)
    bemb = nc.declare_dram_parameter("bemb", [128, 2], F32, isOutput=False)
    batt = nc.declare_dram_parameter("batt", [128, 2], F32, isOutput=False)
    bout = nc.declare_dram_parameter("bout", [1, 1], F32, isOutput=False)
    s_out = nc.declare_dram_parameter("s_out", [1, NLOC * C], F32, isOutput=True)

    NC2 = NLOC * C  # 512: (note, c) column block

    with tile.TileContext(nc) as tc:
        with (
            tc.tile_pool(name="const", bufs=1) as constp,
            tc.tile_pool(name="wt", bufs=4) as wtp,
            tc.tile_pool(name="et", bufs=4) as etp,
            tc.tile_pool(name="mov", bufs=4) as movp,
            tc.tile_pool(name="post", bufs=1) as postp,
            tc.tile_pool(name="psum", bufs=1, space="PSUM") as psp,
        ):
            sc = constp.tile([128, NLOC * J], F32)
            nc.sync.dma_start(sc[:], scales[:])
            wat = constp.tile([128, 2 * K], BF16)
            nc.sync.dma_start(wat[:, 0:K], watT[0])
            nc.sync.dma_start(wat[:, K : 2 * K], watT[1])
            wout = constp.tile([128, 2], F32)
            nc.sync.dma_start(wout[:], woutT[:])
            be = constp.tile([128, 2], F32)
            nc.sync.dma_start(be[:], bemb[:])
            ba = constp.tile([128, 2], F32)
            nc.sync.dma_start(ba[:], batt[:])
            bo = constp.tile([1, 1], F32)
            nc.sync.dma_start(bo[:], bout[:])

            # Warmup reads: the activation engine only supports a single
            # sync-wait per instruction, so let ACT/DVE observe the constant
            # DMA semaphore lanes up front, one lane per tiny instruction.
            warm0 = constp.tile([128, 1], F32)
            nc.scalar.copy(warm0[:], be[:, 0:1])
            warm1 = constp.tile([128, 1], F32)
            nc.scalar.copy(warm1[:], ba[:, 0:1])
            warm2 = constp.tile([1, 1], F32)
            nc.scalar.copy(warm2[:], bo[:])
            warmd = constp.tile([128, 1], F32)
            nc.vector.tensor_copy(warmd[:], sc[:, 0:1])

            # e^T accumulators: [k-half 128, (note,c) 512] fp32, one bank each
            e_ps = [
                psp.tile([128, NC2], F32, name=f"e_ps{m}", tag=f"e_ps{m}")
                for m in range(2)
            ]

            for j in range(J):
                wt = wtp.tile([128, C], BF16)
                nc.sync.dma_start(wt[:], wikiT[j])
                et = etp.tile([128, K], BF16)
                nc.sync.dma_start(et[:], wembT[j])
                mov = movp.tile([128, NC2], BF16)
                # both notes on DVE (ACT has a 1-sync-wait ISA limit and
                # would need waits on both the DMA lane and the WAR release)
                nc.vector.tensor_scalar_mul(mov[:, 0:C], wt[:], sc[:, j : j + 1])
                nc.vector.tensor_scalar_mul(
                    mov[:, C : 2 * C], wt[:], sc[:, J + j : J + j + 1]
                )
                st, sp = (j == 0), (j == J - 1)
                for m in range(2):
                    nc.tensor.matmul(
                        e_ps[m][:],
                        et[:, m * 128 : (m + 1) * 128],
                        mov[:],
                        start=st,
                        stop=sp,
                    )

            # ---- phase 2: bias, attn logits, sigmoid, gate, W_out ----
            ef = []
            eb = []
            for m in range(2):
                ef_m = postp.tile([128, NC2], F32, tag=f"ef{m}")
                nc.scalar.activation(
                    ef_m[:],
                    e_ps[m][:],
                    mybir.ActivationFunctionType.Identity,
                    bias=be[:, m : m + 1],
                    scale=1.0,
                )
                eb_m = postp.tile([128, NC2], BF16, tag=f"eb{m}")
                nc.vector.tensor_copy(eb_m[:], ef_m[:])
                ef.append(ef_m)
                eb.append(eb_m)

            a_ps = [
                psp.tile([128, NC2], F32, name=f"a_ps{jm}", tag=f"a_ps{jm}")
                for jm in range(2)
            ]
            for kt in range(2):
                for jm in range(2):
                    nc.tensor.matmul(
                        a_ps[jm][:],
                        wat[:, kt * K + jm * 128 : kt * K + (jm + 1) * 128],
                        eb[kt][:],
                        start=(kt == 0),
                        stop=(kt == 1),
                    )

            v = []
            for jm in range(2):
                atn = postp.tile([128, NC2], F32, tag=f"atn{jm}")
                nc.scalar.activation(
                    atn[:],
                    a_ps[jm][:],
                    mybir.ActivationFunctionType.Sigmoid,
                    bias=ba[:, jm : jm + 1],
                    scale=1.0,
                )
                v_jm = postp.tile([128, NC2], F32, tag=f"v{jm}")
                nc.vector.tensor_mul(v_jm[:], atn[:], ef[jm][:])
                v.append(v_jm)

            s_ps = psp.tile([1, NC2], F32, tag="s_ps")
            for kt in range(2):
                nc.tensor.matmul(
                    s_ps[:],
                    wout[:, kt : kt + 1],
                    v[kt][:],
                    start=(kt == 0),
                    stop=(kt == 1),
                )
            s_sb = postp.tile([1, NC2], F32, tag="s_sb")
            nc.scalar.activation(
                s_sb[:],
                s_ps[:],
                mybir.ActivationFunctionType.Identity,
                bias=bo[0:1, 0:1],
                scale=1.0,
            )
            nc.sync.dma_start(s_out[:], s_sb[:])

    nc.compile()
    return nc


def _get_nc():
    if "nc" not in _NC_CACHE:
        _NC_CACHE["nc"] = _build_nc()
    return _NC_CACHE["nc"]


def _pad_T_tile(a):
    """(rows, V) -> zero-padded (J, 128, rows') transposed tiles in bf16."""
    rows = a.shape[0]
    out = np.zeros((VP, rows), np.float32)
    out[:V] = a.T
    return np.ascontiguousarray(out.reshape(J, 128, rows)).astype(BF16_NP)


def prep_inputs(notevec, wikivec, W_emb, b_emb, W_att, b_att, W_out, b_out):
    wikiT = _pad_T_tile(np.asarray(wikivec, np.float32))
    wembT = _pad_T_tile(np.asarray(W_emb, np.float32))
    watT = np.ascontiguousarray(
        np.asarray(W_att, np.float32).T.reshape(2, 128, K)
    ).astype(BF16_NP)
    woutT = np.ascontiguousarray(
        np.asarray(W_out, np.float32)[0].reshape(2, 128).T
    )
    bemb = np.ascontiguousarray(np.asarray(b_emb, np.float32).reshape(2, 128).T)
    batt = np.ascontiguousarray(np.asarray(b_att, np.float32).reshape(2, 128).T)
    bout = np.asarray(b_out, np.float32).reshape(1, 1)

    nv = np.zeros((N, VP), np.float32)
    nv[:, :V] = np.asarray(notevec, np.float32)
    in_maps = []
    for i in range(N_CORES):
        # scales[p, l*J + j] = notevec[2i+l, j*128+p]
        sc = np.ascontiguousarray(
            nv[i * NLOC : (i + 1) * NLOC].reshape(NLOC, J, 128).transpose(2, 0, 1)
        ).reshape(128, NLOC * J)
        in_maps.append(
            {
                "wikiT": wikiT,
                "wembT": wembT,
                "scales": np.ascontiguousarray(sc),
                "watT": watT,
                "woutT": woutT,
                "bemb": bemb,
                "batt": batt,
                "bout": bout,
            }
        )
    return in_maps


def run(in_maps, **kw):
    nc = _get_nc()
    return run_bass_kernel_spmd(nc, in_maps, list(range(N_CORES)), **kw)


def kernel(notevec, wikivec, W_emb, b_emb, W_att, b_att, W_out, b_out):
    in_maps = prep_inputs(
        notevec, wikivec, W_emb, b_emb, W_att, b_att, W_out, b_out
    )
    res = run(in_maps)
    out = np.concatenate(
        [r["s_out"].reshape(NLOC, C) for r in res.results], axis=0
    )
    return out.astype(np.float32)
